# revision 10
# baseline (speedup 1.0000x reference)
"""Trainium2 Bass kernel for nn_PositionNet (GLIGEN PositionNet-style module).

Computes, for inputs boxes [B,N,4], masks [B,N], positive_embeddings [B,N,768]
plus MLP params:
  objs  [B, N, 768]        tiny Fourier+MLP head
  sim64 [B, 4126, 4126]    rasterized box similarity mask at 64x64
  sim32 [B, 1054, 1054]    rasterized box similarity mask at 32x32

Sharding: pure data parallel over (batch, row-half): core = 2*b + h.
Each core rasterizes batch b's boxes on-device, computes its half of the
sim masks (rows) against the full pixel set, plus the (duplicated) MLP.
Per-core behavior differences are driven purely by per-core constant
inputs (row-coordinate grids), so one SPMD program serves all 8 cores.
"""

import numpy as np


def _ensure_path():
    try:
        import concourse  # noqa: F401
    except ImportError:
        import sys

        for p in ("/opt/trn_rl_repo", "/root/.axon_site/_ro/trn_rl_repo"):
            if p not in sys.path:
                sys.path.insert(0, p)


_ensure_path()

import concourse.bass as bass  # noqa: E402
import concourse.bacc as bacc  # noqa: E402
import concourse.mybir as mybir  # noqa: E402
import concourse.tile as tile  # noqa: E402
from concourse.masks import make_identity  # noqa: E402

AF = mybir.ActivationFunctionType
OP = mybir.AluOpType
F32 = mybir.dt.float32
BF16 = mybir.dt.bfloat16

B = 4
N = 30
S64, S32 = 64, 32
HW64, HW32 = S64 * S64, S32 * S32  # 4096, 1024
R64, R32 = HW64 // 2, HW32 // 2  # rows per core slab: 2048, 512
NPAD = 896  # 832 padded up to 7*128
TWO_PI = float(2.0 * np.pi)
PI = float(np.pi)
HALF_PI = float(np.pi / 2.0)
RND = float(2.0**23)  # fp32 round-to-nearest-even trick constant


def _emit_raster(nc, pools, size, g_sb, gr_sb, c_sb, valid_sb, nrow_blk):
    """Build am_full [N, size*size] and am_rows [N, nrow_blk*size] (bf16).

    g_sb:  [N, size] pixel coordinate grid (same for x and y)
    gr_sb: [N, nrow_blk] per-core row-block y coordinates
    c_sb:  [N, 4] rounded box corners (x1,y1,x2,y2) scaled to `size`
    valid_sb: [N, 1] 1.0 where the box is rasterized
    """
    cpool, wpool, ampool, pssmall = pools
    x1 = c_sb[:, 0:1]
    y1 = c_sb[:, 1:2]
    x2 = c_sb[:, 2:3]
    y2 = c_sb[:, 3:4]

    t1 = wpool.tile([N, size], F32, tag=f"t1_{size}")
    t2 = wpool.tile([N, size], F32, tag=f"t2_{size}")
    t3 = wpool.tile([N, size], F32, tag=f"t3_{size}")
    xin = wpool.tile([N, size], BF16, tag=f"xin_{size}")
    yv = wpool.tile([N, size], BF16, tag=f"yv_{size}")

    nc.vector.tensor_scalar(out=t1, in0=g_sb, scalar1=x1, scalar2=None, op0=OP.is_ge)
    nc.vector.tensor_scalar(out=t2, in0=g_sb, scalar1=x2, scalar2=None, op0=OP.is_lt)
    nc.vector.tensor_tensor(out=xin, in0=t1, in1=t2, op=OP.mult)

    nc.vector.tensor_scalar(out=t1, in0=g_sb, scalar1=y1, scalar2=None, op0=OP.is_ge)
    nc.vector.tensor_scalar(out=t2, in0=g_sb, scalar1=y2, scalar2=None, op0=OP.is_lt)
    nc.vector.tensor_tensor(out=t3, in0=t1, in1=t2, op=OP.mult)
    nc.vector.tensor_scalar(out=yv, in0=t3, scalar1=valid_sb[:, 0:1], scalar2=None, op0=OP.mult)

    tr1 = wpool.tile([N, nrow_blk], F32, tag=f"tr1_{size}")
    tr2 = wpool.tile([N, nrow_blk], F32, tag=f"tr2_{size}")
    tr3 = wpool.tile([N, nrow_blk], F32, tag=f"tr3_{size}")
    yvr = wpool.tile([N, nrow_blk], BF16, tag=f"yvr_{size}")
    nc.vector.tensor_scalar(out=tr1, in0=gr_sb, scalar1=y1, scalar2=None, op0=OP.is_ge)
    nc.vector.tensor_scalar(out=tr2, in0=gr_sb, scalar1=y2, scalar2=None, op0=OP.is_lt)
    nc.vector.tensor_tensor(out=tr3, in0=tr1, in1=tr2, op=OP.mult)
    nc.vector.tensor_scalar(out=yvr, in0=tr3, scalar1=valid_sb[:, 0:1], scalar2=None, op0=OP.mult)

    hw = size * size
    rows = nrow_blk * size
    am = ampool.tile([N, hw], BF16, tag=f"am_{size}")
    amr = ampool.tile([N, rows], BF16, tag=f"amr_{size}")

    amv = am[:, :].rearrange("p (y x) -> p y x", x=size)
    nc.vector.tensor_tensor(
        out=amv,
        in0=yv[:, :].unsqueeze(2).broadcast_to([N, size, size]),
        in1=xin[:, :].unsqueeze(1).broadcast_to([N, size, size]),
        op=OP.mult,
    )
    amrv = amr[:, :].rearrange("p (y x) -> p y x", x=size)
    nc.vector.tensor_tensor(
        out=amrv,
        in0=yvr[:, :].unsqueeze(2).broadcast_to([N, nrow_blk, size]),
        in1=xin[:, :].unsqueeze(1).broadcast_to([N, nrow_blk, size]),
        op=OP.mult,
    )

    # Null slot: row N-1 = 1 where no valid box covers the pixel. Compute
    # engines can't write at partition base 29, so build the row on
    # partition 0 and DMA it into place.
    ones = cpool.tile([N, 1], BF16, tag="ones")
    nc.vector.memset(ones, 1.0)
    for buf, width, nm in ((am, hw, "nr_am"), (amr, rows, "nr_amr")):
        nullrow = wpool.tile([1, width], BF16, tag=f"{nm}_{size}")
        for c in range(0, width, 512):
            w = min(512, width - c)
            ps = pssmall.tile([1, 512], F32, tag="small")
            nc.tensor.matmul(ps[:, :w], ones, buf[:, c : c + w], start=True, stop=True)
            nc.vector.tensor_scalar(
                out=nullrow[0:1, c : c + w],
                in0=ps[0:1, :w],
                scalar1=0.0,
                scalar2=None,
                op0=OP.is_equal,
            )
        nc.sync.dma_start(out=buf[N - 1 : N, :], in_=nullrow[0:1, :])
    return am, amr


def _build_nc():
    nc = bacc.Bacc("TRN2")

    def din(name, shape):
        return nc.dram_tensor(name, shape, F32, kind="ExternalInput")

    boxes = din("boxes", [N, 4])
    masks = din("masks", [N, 1])
    lt29 = din("lt29", [N, 1])
    g64 = din("g64", [1, S64])
    g32 = din("g32", [1, S32])
    gr64 = din("gr64", [1, S64 // 2])
    gr32 = din("gr32", [1, S32 // 2])
    pemb = din("pemb", [N, 768])
    ptab = din("ptab", [99, 768])
    nullpos = din("nullpos", [1, 768])
    nullxy = din("nullxy", [1, 64])
    freqs = din("freqs", [1, 32])
    w1 = din("w1", [NPAD, 512])
    w2 = din("w2", [512, 512])
    w3 = din("w3", [512, 768])
    b1 = din("b1", [1, 512])
    b2 = din("b2", [1, 512])
    b3 = din("b3", [1, 768])

    out64 = nc.dram_tensor("out64", [R64, HW64 + N], F32, kind="ExternalOutput")
    out32 = nc.dram_tensor("out32", [R32, HW32 + N], F32, kind="ExternalOutput")
    objs = nc.dram_tensor("objs", [N, 768], F32, kind="ExternalOutput")

    def dbcast(dram_t, parts):
        a = dram_t[0:1, :]
        return a.broadcast_to([parts, a.shape[1]])

    with tile.TileContext(nc) as tc:
        with (
            tc.tile_pool(name="const", bufs=1) as cpool,
            tc.tile_pool(name="work", bufs=1) as wpool,
            tc.tile_pool(name="am", bufs=1) as ampool,
            tc.tile_pool(name="outb", bufs=3) as opool,
            tc.tile_pool(name="psmm", bufs=5, space="PSUM") as psmm,
            tc.tile_pool(name="pssmall", bufs=3, space="PSUM") as pssmall,
        ):
            # ---- constants / inputs to SBUF ----
            bx = cpool.tile([N, 4], F32, tag="bx")
            nc.sync.dma_start(out=bx, in_=boxes[:, :])
            mk = cpool.tile([N, 1], F32, tag="mk")
            nc.sync.dma_start(out=mk, in_=masks[:, :])
            l29 = cpool.tile([N, 1], F32, tag="l29")
            nc.sync.dma_start(out=l29, in_=lt29[:, :])
            gg64 = cpool.tile([N, S64], F32, tag="gg64")
            nc.sync.dma_start(out=gg64, in_=dbcast(g64, N))
            gg32 = cpool.tile([N, S32], F32, tag="gg32")
            nc.sync.dma_start(out=gg32, in_=dbcast(g32, N))
            ggr64 = cpool.tile([N, S64 // 2], F32, tag="ggr64")
            nc.sync.dma_start(out=ggr64, in_=dbcast(gr64, N))
            ggr32 = cpool.tile([N, S32 // 2], F32, tag="ggr32")
            nc.sync.dma_start(out=ggr32, in_=dbcast(gr32, N))

            # ---- rounded corners: RNE via +/- 2^23 (matches np.round) ----
            c64 = wpool.tile([N, 4], F32, tag="c64")
            nc.vector.tensor_scalar(out=c64, in0=bx, scalar1=float(S64), scalar2=RND, op0=OP.mult, op1=OP.add)
            nc.vector.tensor_scalar(out=c64, in0=c64, scalar1=RND, scalar2=None, op0=OP.subtract)
            c32 = wpool.tile([N, 4], F32, tag="c32")
            nc.vector.tensor_scalar(out=c32, in0=bx, scalar1=float(S32), scalar2=RND, op0=OP.mult, op1=OP.add)
            nc.vector.tensor_scalar(out=c32, in0=c32, scalar1=RND, scalar2=None, op0=OP.subtract)

            # ---- valid = (mask == 1) & (n < N-1) ----
            meq = wpool.tile([N, 1], F32, tag="meq")
            nc.vector.tensor_scalar(out=meq, in0=mk, scalar1=1.0, scalar2=None, op0=OP.is_equal)
            valid = wpool.tile([N, 1], F32, tag="valid")
            nc.vector.tensor_tensor(out=valid, in0=meq, in1=l29, op=OP.mult)

            pools = (cpool, wpool, ampool, pssmall)
            am64, amr64 = _emit_raster(nc, pools, S64, gg64, ggr64, c64, valid, S64 // 2)
            am32, amr32 = _emit_raster(nc, pools, S32, gg32, ggr32, c32, valid, S32 // 2)

            identb = cpool.tile([N, N], BF16, tag="identb")
            make_identity(nc, identb[:, :])
            identf = cpool.tile([N, N], F32, tag="identf")
            make_identity(nc, identf[:, :])

            # ---- sim64 rows: out[t*128+i, :] ----
            for t in range(R64 // 128):
                lhsT = amr64[:, t * 128 : (t + 1) * 128]
                ob = opool.tile([128, HW64 + N], F32, tag="ob64")
                for c in range(HW64 // 512):
                    ps = psmm.tile([128, 512], F32, tag="mm")
                    nc.tensor.matmul(ps, lhsT, am64[:, c * 512 : (c + 1) * 512], start=True, stop=True)
                    nc.vector.tensor_scalar(
                        out=ob[:, c * 512 : (c + 1) * 512], in0=ps, scalar1=1.0, scalar2=None, op0=OP.min
                    )
                tp = psmm.tile([128, N], BF16, tag="mm")
                nc.tensor.transpose(tp, lhsT, identb[:, :])
                nc.scalar.copy(out=ob[:, HW64 : HW64 + N], in_=tp)
                nc.sync.dma_start(out=out64[t * 128 : (t + 1) * 128, :], in_=ob)

            # ---- sim32 rows ----
            for t in range(R32 // 128):
                lhsT = amr32[:, t * 128 : (t + 1) * 128]
                ob = opool.tile([128, HW32 + N], F32, tag="ob32")
                for c in range(HW32 // 512):
                    ps = psmm.tile([128, 512], F32, tag="mm")
                    nc.tensor.matmul(ps, lhsT, am32[:, c * 512 : (c + 1) * 512], start=True, stop=True)
                    nc.vector.tensor_scalar(
                        out=ob[:, c * 512 : (c + 1) * 512], in0=ps, scalar1=1.0, scalar2=None, op0=OP.min
                    )
                tp = psmm.tile([128, N], BF16, tag="mm")
                nc.tensor.transpose(tp, lhsT, identb[:, :])
                nc.scalar.copy(out=ob[:, HW32 : HW32 + N], in_=tp)
                nc.sync.dma_start(out=out32[t * 128 : (t + 1) * 128, :], in_=ob)

            # ---- MLP head ----
            ptab_sb = cpool.tile([N, 768], F32, tag="ptab")
            nc.sync.dma_start(out=ptab_sb, in_=ptab[0:N, :])
            pemb_sb = cpool.tile([N, 768], F32, tag="pemb")
            nc.sync.dma_start(out=pemb_sb, in_=pemb[:, :])
            npos_sb = cpool.tile([N, 768], F32, tag="npos")
            nc.sync.dma_start(out=npos_sb, in_=dbcast(nullpos, N))
            nxy_sb = cpool.tile([N, 64], F32, tag="nxy")
            nc.sync.dma_start(out=nxy_sb, in_=dbcast(nullxy, N))
            fr_sb = cpool.tile([N, 32], F32, tag="fr")
            nc.sync.dma_start(out=fr_sb, in_=dbcast(freqs, N))
            b1_sb = cpool.tile([N, 512], F32, tag="b1")
            nc.sync.dma_start(out=b1_sb, in_=dbcast(b1, N))
            b2_sb = cpool.tile([N, 512], F32, tag="b2")
            nc.sync.dma_start(out=b2_sb, in_=dbcast(b2, N))
            b3_sb = cpool.tile([N, 768], F32, tag="b3")
            nc.sync.dma_start(out=b3_sb, in_=dbcast(b3, N))
            w1_sb = cpool.tile([128, NPAD // 128, 512], F32, tag="w1")
            nc.sync.dma_start(out=w1_sb, in_=w1[:, :].rearrange("(k p) n -> p k n", p=128))
            w2_sb = cpool.tile([128, 4, 512], F32, tag="w2")
            nc.sync.dma_start(out=w2_sb, in_=w2[:, :].rearrange("(k p) n -> p k n", p=128))
            w3_sb = cpool.tile([128, 4, 768], F32, tag="w3")
            nc.sync.dma_start(out=w3_sb, in_=w3[:, :].rearrange("(k p) n -> p k n", p=128))

            hpib = cpool.tile([N, 1], F32, tag="hpib")
            nc.vector.memset(hpib, HALF_PI)
            zb = cpool.tile([N, 1], F32, tag="zb")
            nc.vector.memset(zb, 0.0)

            om = wpool.tile([N, 1], F32, tag="om")  # 1 - mask
            nc.vector.tensor_scalar(out=om, in0=mk, scalar1=-1.0, scalar2=1.0, op0=OP.mult, op1=OP.add)

            h = wpool.tile([N, NPAD], F32, tag="h")
            # pe = (pemb + ptab) * m + (1-m) * null_positive
            pe = wpool.tile([N, 768], F32, tag="pe")
            nc.vector.tensor_tensor(out=pe, in0=pemb_sb, in1=ptab_sb, op=OP.add)
            nc.vector.tensor_scalar(out=pe, in0=pe, scalar1=mk[:, 0:1], scalar2=None, op0=OP.mult)
            tmp768 = wpool.tile([N, 768], F32, tag="tmp768")
            nc.vector.tensor_scalar(out=tmp768, in0=npos_sb, scalar1=om[:, 0:1], scalar2=None, op0=OP.mult)
            nc.vector.tensor_tensor(out=h[:, 0:768], in0=pe, in1=tmp768, op=OP.add)

            # Fourier: for f, c: sin(freq_f * box_c), cos(freq_f * box_c)
            bt = wpool.tile([N, 32], F32, tag="bt")
            btv = bt[:, :].rearrange("p (f c) -> p f c", c=4)
            nc.vector.tensor_copy(out=btv, in_=bx[:, :].unsqueeze(1).broadcast_to([N, 8, 4]))
            nc.vector.tensor_tensor(out=bt, in0=bt, in1=fr_sb, op=OP.mult)  # fx
            # Range-reduce for ACT Sin (valid domain [-pi, pi]):
            #   r = fx - 2*pi*round(fx / 2*pi)  in [-pi, pi]; sin(fx) = sin(r)
            # round() via the +2^23 fp32 RNE trick. For cos, round((fx+pi/2)/2pi)
            # and add the pi/2 phase via the activation bias.
            inv2pi = 1.0 / TWO_PI
            tsin = wpool.tile([N, 32], F32, tag="tsin")
            nc.vector.tensor_scalar(out=tsin, in0=bt, scalar1=inv2pi, scalar2=RND, op0=OP.mult, op1=OP.add)
            nc.vector.tensor_scalar(out=tsin, in0=tsin, scalar1=RND, scalar2=None, op0=OP.subtract)
            rsin = wpool.tile([N, 32], F32, tag="rsin")
            nc.vector.scalar_tensor_tensor(out=rsin, in0=tsin, scalar=-TWO_PI, in1=bt, op0=OP.mult, op1=OP.add)
            tcos = wpool.tile([N, 32], F32, tag="tcos")
            nc.vector.tensor_scalar(out=tcos, in0=bt, scalar1=inv2pi, scalar2=0.25, op0=OP.mult, op1=OP.add)
            nc.vector.tensor_scalar(out=tcos, in0=tcos, scalar1=RND, scalar2=RND, op0=OP.add, op1=OP.subtract)
            rcos = wpool.tile([N, 32], F32, tag="rcos")
            nc.vector.scalar_tensor_tensor(out=rcos, in0=tcos, scalar=-TWO_PI, in1=bt, op0=OP.mult, op1=OP.add)
            xy = wpool.tile([N, 64], F32, tag="xy")
            xyv = xy[:, :].rearrange("p (f c) -> p f c", c=8)
            nc.scalar.activation(
                out=xyv[:, :, 0:4], in_=rsin[:, :].rearrange("p (f c) -> p f c", c=4),
                func=AF.Sin, bias=zb[:, 0:1], scale=1.0,
            )
            nc.scalar.activation(
                out=xyv[:, :, 4:8], in_=rcos[:, :].rearrange("p (f c) -> p f c", c=4),
                func=AF.Sin, bias=hpib[:, 0:1], scale=1.0,
            )
            nc.vector.tensor_scalar(out=xy, in0=xy, scalar1=mk[:, 0:1], scalar2=None, op0=OP.mult)
            tmp64 = wpool.tile([N, 64], F32, tag="tmp64")
            nc.vector.tensor_scalar(out=tmp64, in0=nxy_sb, scalar1=om[:, 0:1], scalar2=None, op0=OP.mult)
            nc.vector.tensor_tensor(out=h[:, 768:832], in0=xy, in1=tmp64, op=OP.add)
            nc.vector.memset(h[:, 832:NPAD], 0.0)

            def transpose_to(h_src, n_k, tag):
                hT = wpool.tile([128, n_k * N], F32, tag=tag)
                for k in range(n_k):
                    tp = psmm.tile([128, N], F32, tag="mm")
                    nc.tensor.transpose(tp, h_src[:, k * 128 : (k + 1) * 128], identf[:, :])
                    nc.scalar.copy(out=hT[:, k * N : (k + 1) * N], in_=tp)
                return hT

            hT = transpose_to(h, NPAD // 128, "hT")
            ps1 = pssmall.tile([N, 512], F32, tag="small")
            for k in range(NPAD // 128):
                nc.tensor.matmul(
                    ps1, hT[:, k * N : (k + 1) * N], w1_sb[:, k, :],
                    start=(k == 0), stop=(k == NPAD // 128 - 1),
                )
            h1 = wpool.tile([N, 512], F32, tag="h1")
            nc.vector.tensor_tensor(out=h1, in0=ps1, in1=b1_sb, op=OP.add)
            h1g = wpool.tile([N, 512], F32, tag="h1g")
            nc.scalar.activation(out=h1g, in_=h1, func=AF.Sigmoid, bias=zb[:, 0:1])
            h1s = wpool.tile([N, 512], F32, tag="h1s")
            nc.vector.tensor_tensor(out=h1s, in0=h1, in1=h1g, op=OP.mult)

            h1T = transpose_to(h1s, 4, "h1T")
            ps2 = pssmall.tile([N, 512], F32, tag="small")
            for k in range(4):
                nc.tensor.matmul(
                    ps2, h1T[:, k * N : (k + 1) * N], w2_sb[:, k, :],
                    start=(k == 0), stop=(k == 3),
                )
            h2 = wpool.tile([N, 512], F32, tag="h2")
            nc.vector.tensor_tensor(out=h2, in0=ps2, in1=b2_sb, op=OP.add)
            h2g = wpool.tile([N, 512], F32, tag="h2g")
            nc.scalar.activation(out=h2g, in_=h2, func=AF.Sigmoid, bias=zb[:, 0:1])
            h2s = wpool.tile([N, 512], F32, tag="h2s")
            nc.vector.tensor_tensor(out=h2s, in0=h2, in1=h2g, op=OP.mult)

            h2T = transpose_to(h2s, 4, "h2T")
            ps3a = pssmall.tile([N, 512], F32, tag="small")
            ps3b = pssmall.tile([N, 256], F32, tag="small")
            for k in range(4):
                nc.tensor.matmul(
                    ps3a, h2T[:, k * N : (k + 1) * N], w3_sb[:, k, 0:512],
                    start=(k == 0), stop=(k == 3),
                )
            for k in range(4):
                nc.tensor.matmul(
                    ps3b, h2T[:, k * N : (k + 1) * N], w3_sb[:, k, 512:768],
                    start=(k == 0), stop=(k == 3),
                )
            ob = wpool.tile([N, 768], F32, tag="objs")
            nc.vector.tensor_tensor(out=ob[:, 0:512], in0=ps3a, in1=b3_sb[:, 0:512], op=OP.add)
            nc.vector.tensor_tensor(out=ob[:, 512:768], in0=ps3b, in1=b3_sb[:, 512:768], op=OP.add)
            nc.sync.dma_start(out=objs[:, :], in_=ob)

    nc.finalize()
    return nc


_CACHE = {}


def get_nc():
    if "nc" not in _CACHE:
        _CACHE["nc"] = _build_nc()
    return _CACHE["nc"]


def make_in_map(core, boxes, masks, positive_embeddings, pos_table, W1, b1, W2, b2, W3, b3,
                null_positive, null_position):
    f32 = np.float32
    b, h = core // 2, core % 2
    w1p = np.zeros((NPAD, 512), f32)
    w1p[: W1.shape[0]] = W1
    freqs = (100.0 ** (np.arange(8, dtype=np.float64) / 8.0)).astype(f32)
    return dict(
        boxes=np.ascontiguousarray(boxes[b], f32),
        masks=np.ascontiguousarray(masks[b][:, None], f32),
        lt29=(np.arange(N) < N - 1).astype(f32)[:, None],
        g64=np.arange(S64, dtype=f32)[None],
        g32=np.arange(S32, dtype=f32)[None],
        gr64=(np.arange(S64 // 2, dtype=f32) + (S64 // 2) * h)[None],
        gr32=(np.arange(S32 // 2, dtype=f32) + (S32 // 2) * h)[None],
        pemb=np.ascontiguousarray(positive_embeddings[b], f32),
        ptab=np.ascontiguousarray(pos_table, f32),
        nullpos=np.ascontiguousarray(null_positive[None], f32),
        nullxy=np.ascontiguousarray(null_position[None], f32),
        freqs=np.repeat(freqs, 4)[None].astype(f32),
        w1=w1p,
        w2=np.ascontiguousarray(W2, f32),
        w3=np.ascontiguousarray(W3, f32),
        b1=np.ascontiguousarray(b1[None], f32),
        b2=np.ascontiguousarray(b2[None], f32),
        b3=np.ascontiguousarray(b3[None], f32),
    )


def assemble(results):
    f32 = np.float32
    objs = np.zeros((B, N, 768), f32)
    sim64 = np.zeros((B, HW64 + N, HW64 + N), f32)
    sim32 = np.zeros((B, HW32 + N, HW32 + N), f32)
    for core, res in enumerate(results):
        b, h = core // 2, core % 2
        sim64[b, h * R64 : (h + 1) * R64, :] = res["out64"]
        sim32[b, h * R32 : (h + 1) * R32, :] = res["out32"]
        if h == 0:
            objs[b] = res["objs"]
    return objs, sim64, sim32


def kernel(boxes, masks, positive_embeddings, pos_table, W1, b1, W2, b2, W3, b3,
           null_positive, null_position, **run_kwargs):
    from concourse.bass_utils import run_bass_kernel_spmd

    nc = get_nc()
    args = (boxes, masks, positive_embeddings, pos_table, W1, b1, W2, b2, W3, b3,
            null_positive, null_position)
    in_maps = [make_in_map(core, *args) for core in range(2 * B)]
    out = run_bass_kernel_spmd(nc, in_maps, core_ids=list(range(2 * B)), **run_kwargs)
    res = assemble(out.results)
    if run_kwargs:
        _CACHE["last_run"] = out
    return res
